# revision 40
# baseline (speedup 1.0000x reference)
"""Trainium2 Bass kernel for ActorCriticLSTM (T=256, B=1024, O=H=128, A=8).

Strategy: data-parallel over batch (8 cores x 128 batch). Per core:
  - Encoder (2-layer tanh MLP) computed in "transposed activation" layout
    xT [feature, (t, b)] so chained matmuls need no transposes; obs is
    pre-transposed on host and uploaded as one [128, T*B] bf16 matrix.
  - LSTM recurrence fully unrolled over T=256 steps, state kept transposed
    (hT [H,B] bf16 slice of a persistent ys buffer, cT [H,B] f32).
    Gates for step t are 4 PSUM column blocks [H,B] each computed as
    W_ih_g @ x2_t (start) + W_hh_g @ (mask*h) (accumulate).
    Done-masking uses a host-precomputed broadcast mask [T,128,B] bf16.
  - Actor/critic heads + tanh-normal logprob math in natural layout
    [batch, (t,a)] after the scan; Ln-based atanh; outputs left in
    device-natural layouts and fixed up (transposed) on host.
"""

import os
import numpy as np
import ml_dtypes

import concourse.bass as bass
import concourse.bacc as bacc
import concourse.tile as tile
from concourse import mybir
from concourse.bass_utils import run_bass_kernel_spmd

BF16 = mybir.dt.bfloat16
F32 = mybir.dt.float32
AF = mybir.ActivationFunctionType
ALU = mybir.AluOpType

LOG2PI = float(np.log(2.0 * np.pi))
TANH_EPS = 1e-06

T, B, O, H, A = 256, 1024, 128, 128, 8
NCORES = 8
BS = B // NCORES  # 128 batch per core


def build_program(T_=T, CT=4):
    """Build the single-core SPMD Bass program."""
    nc = bacc.Bacc("TRN2", target_bir_lowering=False, debug=False)

    # ---- DRAM I/O ----
    obsT_d = nc.dram_tensor("obsT", [128, T_ * BS], BF16, kind="ExternalInput")
    mask_d = nc.dram_tensor("maskb", [T_, 128, BS], BF16, kind="ExternalInput")
    act_d = nc.dram_tensor("act_nat", [128, T_ * A], F32, kind="ExternalInput")
    h0T_d = nc.dram_tensor("h0T", [H, BS], BF16, kind="ExternalInput")
    c0T_d = nc.dram_tensor("c0T", [H, BS], F32, kind="ExternalInput")
    w1T_d = nc.dram_tensor("w1T", [O, H], BF16, kind="ExternalInput")
    w2T_d = nc.dram_tensor("w2T", [H, H], BF16, kind="ExternalInput")
    wihT_d = nc.dram_tensor("wihT", [H, 4 * H], BF16, kind="ExternalInput")
    whhT_d = nc.dram_tensor("whhT", [H, 4 * H], BF16, kind="ExternalInput")
    amcr_d = nc.dram_tensor("amcrT", [H, A + 1], BF16, kind="ExternalInput")

    lp_d = nc.dram_tensor("lp_nat", [128, T_], F32, kind="ExternalOutput")
    ent_d = nc.dram_tensor("ent_nat", [128, T_], F32, kind="ExternalOutput")
    val_d = nc.dram_tensor("val_nat", [128, T_], F32, kind="ExternalOutput")
    hT_out_d = nc.dram_tensor("hT_out", [H, BS], F32, kind="ExternalOutput")
    cT_out_d = nc.dram_tensor("cT_out", [H, BS], F32, kind="ExternalOutput")

    NCH = T_ // CT
    with tile.TileContext(nc) as tc:
        with (
            tc.tile_pool(name="const", bufs=1) as cpool,
            tc.tile_pool(name="x2all", bufs=1) as x2pool,
            tc.tile_pool(name="ysall", bufs=1) as yspool,
            tc.tile_pool(name="obs", bufs=3) as obspool,
            tc.tile_pool(name="x1", bufs=2) as x1pool,
            tc.tile_pool(name="epsum", bufs=1, space="PSUM") as epsum,
            tc.tile_pool(name="gpsum", bufs=2, space="PSUM") as gpsum,
            tc.tile_pool(name="hpsum", bufs=2, space="PSUM") as hpsum,
            tc.tile_pool(name="mask", bufs=12) as mpool,
            tc.tile_pool(name="work", bufs=3) as wpool,
            tc.tile_pool(name="cstate", bufs=2) as capool,
            tc.tile_pool(name="post", bufs=1) as ppool,
        ):
            # actions DMA first: it gates L1/L2 which head the ACT queue.
            # Chunked halves so the first Ln ops overlap the second DMA half.
            acts = ppool.tile([128, T_ * A], F32, tag="acts")
            L1 = ppool.tile([128, T_ * A], F32, tag="L1")
            L2 = ppool.tile([128, T_ * A], F32, tag="L2")
            half = T_ * A // 2
            for q in range(2):
                qs = slice(q * half, (q + 1) * half)
                nc.sync.dma_start(acts[:, qs], act_d[:, qs])
            for q in range(2):
                qs = slice(q * half, (q + 1) * half)
                nc.scalar.activation(L1[:, qs], acts[:, qs], AF.Ln,
                                     bias=1.0, scale=1.0)
                nc.scalar.activation(L2[:, qs], acts[:, qs], AF.Ln,
                                     bias=1.0, scale=-1.0)

            # ---- constants ----
            w1T = cpool.tile([O, H], BF16, tag="w1T")
            nc.sync.dma_start(w1T[:], w1T_d[:])
            w2T = cpool.tile([H, H], BF16, tag="w2T")
            nc.sync.dma_start(w2T[:], w2T_d[:])
            wihT = cpool.tile([H, 4 * H], BF16, tag="wihT")
            nc.sync.dma_start(wihT[:], wihT_d[:])
            whhT = cpool.tile([H, 4 * H], BF16, tag="whhT")
            nc.sync.dma_start(whhT[:], whhT_d[:])
            amcr = cpool.tile([H, A + 1], BF16, tag="amcr")
            nc.sync.dma_start(amcr[:], amcr_d[:])
            h0T = cpool.tile([H, BS], BF16, tag="h0T")
            nc.sync.dma_start(h0T[:], h0T_d[:])
            c0T = cpool.tile([H, BS], F32, tag="c0T")
            nc.sync.dma_start(c0T[:], c0T_d[:])

            x2all = x2pool.tile([128, T_ * BS], BF16, tag="x2all")
            ysall = yspool.tile([128, T_ * BS], BF16, tag="ysall")
            mv = ppool.tile([128, T_ * (A + 1)], F32, tag="mv")

            # logprob math is cut into [128,256] pieces and spread through
            # the recurrence (DVE is ~50% idle there); tile reuse:
            # d -> ljbuf (piecewise, after its ljs reduce), z -> L1, zsq -> L2
            ljbuf = ppool.tile([128, T_ * A], F32, tag="ljbuf")
            ljs = ppool.tile([128, T_], F32, tag="ljs")
            zsum = ppool.tile([128, T_], F32, tag="zsum")
            mv3 = mv[:].rearrange("p (t n) -> p t n", n=A + 1)
            NP = max(1, T_ * A // 256)  # pieces of 256 cols = 32 steps
            SP = T_ // NP

            PW = T_ * A // NP

            def pc(x, k):
                return x[:, k * PW:(k + 1) * PW]

            def lj_piece(k):
                nc.vector.tensor_add(pc(ljbuf, k), pc(L1, k), pc(L2, k))

            def ljs_piece(k):
                nc.vector.tensor_reduce(
                    ljs[:, k * SP:(k + 1) * SP],
                    pc(ljbuf, k).rearrange("p (t a) -> p t a", a=A),
                    axis=mybir.AxisListType.X, op=ALU.add)

            def d_piece(k):
                nc.vector.tensor_sub(pc(ljbuf, k), pc(L1, k), pc(L2, k))

            def z_piece(k):
                nc.vector.scalar_tensor_tensor(
                    pc(L1, k).rearrange("p (t a) -> p t a", a=A),
                    pc(ljbuf, k).rearrange("p (t a) -> p t a", a=A), 0.5,
                    mv3[:, k * SP:(k + 1) * SP, 0:A],
                    op0=ALU.mult, op1=ALU.subtract)

            def zsq_piece(k):
                nc.vector.tensor_mul(pc(L2, k), pc(L1, k), pc(L1, k))

            def zsum_piece(k):
                nc.vector.tensor_reduce(
                    zsum[:, k * SP:(k + 1) * SP],
                    pc(L2, k).rearrange("p (t a) -> p t a", a=A),
                    axis=mybir.AxisListType.X, op=ALU.add)

            spread = {}
            for k in range(NP):
                spread.setdefault(8 + 3 * k, []).extend(
                    [lambda k=k: lj_piece(k), lambda k=k: ljs_piece(k),
                     lambda k=k: d_piece(k)])
            for k in range(NP - 1):
                s = SP * (k + 1) + 2
                spread.setdefault(s, []).append(lambda k=k: z_piece(k))
                spread.setdefault(s + 1, []).append(lambda k=k: zsq_piece(k))
                spread.setdefault(s + 2, []).append(lambda k=k: zsum_piece(k))

            # Per-step recurrence state (python-side references):
            #   cm[t]    = c(t-1) * m(t)          (GpSimd, off critical path)
            #   som[t-1] = sigma_o(t-1) * m(t)    (GpSimd, off critical path)
            #   hm[t]    = som[t-1] * th(t-1)     (DVE, critical)
            # Critical cycle: hm -> hh-mms -> sigma_if -> t1 -> cnew -> tanh -> hm
            mtiles = {}

            def mask_tile(t):
                if t not in mtiles and t < T_:
                    m = mpool.tile([128, BS], BF16, tag="mask")
                    nc.sync.dma_start(m[:], mask_d[t])
                    mtiles[t] = m
                return mtiles.get(t)

            cm_t = None      # c(t-1)*m(t) for current step
            th_prev = None   # tanh(c(t-1))
            sig_prev = None  # sigma tile of step t-1 (for sigma_o)
            som_prev = None  # sigma_o(t-1)*m(t)
            gtiles = {}      # step -> gate psum tile (ih-mms pre-issued)

            def issue_ih(t):
                if t >= T_ or t in gtiles:
                    return
                # two banks: (i,f) and (g,o); sigma_if can then start as
                # soon as the i,f hh-mms land, without waiting for g,o
                gif = gpsum.tile([128, 256], F32, tag="gif")
                ggo = gpsum.tile([128, 256], F32, tag="ggo")
                x2t = x2all[:, t * BS:(t + 1) * BS]
                for gi in range(4):
                    # start=True on the first mm zero-marks the whole bank;
                    # the second ih-mm overwrites its (pending-zero) slice,
                    # hh-mms then accumulate; last hh-mm closes the group.
                    gt = gif if gi < 2 else ggo
                    nc.tensor.matmul(gt[:, (gi % 2) * 128:(gi % 2 + 1) * 128],
                                     wihT[:, gi * 128:(gi + 1) * 128], x2t,
                                     start=(gi % 2 == 0), stop=False)
                gtiles[t] = (gif, ggo)

            enc_state = {}

            def encoder_piece(c, phase, after=None):
                # one piece per recurrence step: phase 0 = obs DMA,
                # 1 = layer-1 mm+tanh, 2 = layer-2 mm+tanh (CT*BS == 512)
                if c >= NCH or phase > 2:
                    return
                if phase == 0:
                    obst = obspool.tile([128, CT * BS], BF16, tag="obs")
                    nc.gpsimd.dma_start(
                        obst[:], obsT_d[:, c * CT * BS:(c + 1) * CT * BS])
                    enc_state[c] = obst
                    return
                # negative priority offset: encoder work appears ~2 steps
                # later to the scheduler, so its tanh lands in ACT idle gaps
                # instead of between chain ACT ops (hard dep-pinning stalls
                # the in-order ACT queue; this is only a placement hint)
                if phase == 1:
                    obst = enc_state[c]
                    x1 = x1pool.tile([128, CT * BS], BF16, tag="x1")
                    with tc.high_priority(offset=-60):
                        ep = epsum.tile([128, 512], F32, tag="ep")
                        nc.tensor.matmul(ep[:], w1T[:], obst[:],
                                         start=True, stop=True)
                        nc.scalar.activation(x1[:], ep[:], AF.Tanh)
                    enc_state[c] = x1
                else:
                    x1 = enc_state.pop(c)
                    with tc.high_priority(offset=-60):
                        ep2 = epsum.tile([128, 512], F32, tag="ep")
                        nc.tensor.matmul(ep2[:], w2T[:], x1[:],
                                         start=True, stop=True)
                        nc.scalar.activation(
                            x2all[:, c * CT * BS:(c + 1) * CT * BS],
                            ep2[:], AF.Tanh)
                del after

            def encoder_chunk(c):
                for ph in range(3):
                    encoder_piece(c, ph)

            cprev = c0T
            # encoder runs one chunk ahead of the recurrence so the ih-mm
            # lookahead never stalls at chunk boundaries
            encoder_chunk(0)
            encoder_chunk(1)
            issue_ih(0)
            for c in range(NCH):
                # ---------- recurrence steps ----------
                for t in range(c * CT, (c + 1) * CT):
                    m_cur = mask_tile(t)
                    m_next = mask_tile(t + 1)
                    mask_tile(t + 6)  # deepen DMA prefetch
                    issue_ih(t + 1)
                    if t == 0:
                        hm = wpool.tile([H, BS], BF16, tag="hm")
                        nc.vector.tensor_mul(hm[:], h0T[:], m_cur[:])
                        cm_t = wpool.tile([H, BS], F32, tag="cm0")
                        nc.gpsimd.tensor_mul(cm_t[:], c0T[:], m_cur[:])
                    else:
                        # hm = (sigma_o(t-1)*m(t)) * tanh(c(t-1))  [critical]
                        hm = wpool.tile([H, BS], BF16, tag="hm")
                        nc.vector.tensor_mul(hm[:], som_prev[:], th_prev[:])
                        # ys[t-1] = sigma_o(t-1) * tanh(c(t-1))    [off-path]
                        ysl_prev = ysall[:, (t - 1) * BS:t * BS]
                        nc.vector.tensor_mul(ysl_prev, sig_prev[:, 384:512],
                                             th_prev[:])
                    gif, ggo = gtiles.pop(t)
                    for gi in range(4):
                        gt = gif if gi < 2 else ggo
                        nc.tensor.matmul(gt[:, (gi % 2) * 128:(gi % 2 + 1) * 128],
                                         whhT[:, gi * 128:(gi + 1) * 128], hm[:],
                                         start=False, stop=(gi % 2 == 1))
                    if t > 0:
                        # heads for t-1, after the critical hh-mms
                        hp = hpsum.tile([128, A + 1], F32, tag="hp")
                        nc.tensor.matmul(hp[:], ysall[:, (t - 1) * BS:t * BS],
                                         amcr[:], start=True, stop=True)
                        nc.vector.tensor_copy(
                            mv[:, (t - 1) * (A + 1):t * (A + 1)], hp[:])
                    # gate order i,f,g,o (reference order)
                    sig = wpool.tile([128, 4 * H], BF16, tag="sig")
                    nc.scalar.activation(sig[:, 0:256], gif[:], AF.Sigmoid)
                    tg = wpool.tile([128, H], BF16, tag="tg")
                    nc.scalar.activation(tg[:], ggo[:, 0:128], AF.Tanh)
                    nc.scalar.activation(sig[:, 384:512], ggo[:, 128:256], AF.Sigmoid)
                    # c-path on DVE: t2 first (frees the cnew issue as soon
                    # as t1 lands), t1 after tanh_g
                    t2 = wpool.tile([H, BS], F32, tag="t2")
                    nc.vector.tensor_mul(t2[:], sig[:, 128:256], cm_t[:])
                    t1 = wpool.tile([H, BS], BF16, tag="t1")
                    nc.vector.tensor_mul(t1[:], sig[:, 0:128], tg[:])
                    cnew = capool.tile([H, BS], F32, tag="c")
                    nc.vector.tensor_add(cnew[:], t1[:], t2[:])
                    th = wpool.tile([H, BS], BF16, tag="th")
                    th_i = nc.scalar.activation(th[:], cnew[:], AF.Tanh)
                    if t + 1 < T_:
                        cm_t = wpool.tile([H, BS], F32, tag="cmn")
                        nc.gpsimd.tensor_mul(cm_t[:], cnew[:], m_next[:])
                        som_prev = wpool.tile([H, BS], BF16, tag="som")
                        nc.gpsimd.tensor_mul(som_prev[:], sig[:, 384:512],
                                             m_next[:])
                    th_prev = th
                    sig_prev = sig
                    cprev = cnew
                    encoder_piece(c + 2, t % CT, after=th_i)
                    for fn in spread.pop(t, []):
                        fn()

            # tail: ys / heads for the last step
            ysl_last = ysall[:, (T_ - 1) * BS:T_ * BS]
            nc.vector.tensor_mul(ysl_last, sig_prev[:, 384:512], th_prev[:])
            hp = hpsum.tile([128, A + 1], F32, tag="hp")
            nc.tensor.matmul(hp[:], ysl_last, amcr[:], start=True, stop=True)
            nc.vector.tensor_copy(mv[:, (T_ - 1) * (A + 1):T_ * (A + 1)], hp[:])

            # ---------- final state outputs ----------
            nc.sync.dma_start(cT_out_d[:], cprev[:])
            hT_f32 = ppool.tile([H, BS], F32, tag="hTf")
            nc.vector.tensor_copy(hT_f32[:], ysall[:, (T_ - 1) * BS:T_ * BS])
            nc.sync.dma_start(hT_out_d[:], hT_f32[:])

            # ---------- logprob finals (last piece + reductions) ----------
            z_piece(NP - 1)
            zsq_piece(NP - 1)
            zsum_piece(NP - 1)
            lp = ppool.tile([128, T_], F32, tag="lp")
            nc.vector.scalar_tensor_tensor(lp[:], zsum[:], -0.5, ljs[:],
                                           op0=ALU.mult, op1=ALU.subtract)
            lpf = ppool.tile([128, T_], F32, tag="lpf")
            nc.vector.tensor_scalar_add(lpf[:], lp[:], -0.5 * A * LOG2PI)
            nc.sync.dma_start(lp_d[:], lpf[:])
            ent = ppool.tile([128, T_], F32, tag="ent")
            nc.vector.tensor_scalar_add(ent[:], ljs[:], A * (0.5 + 0.5 * LOG2PI))
            nc.sync.dma_start(ent_d[:], ent[:])
            vals = ppool.tile([128, T_], F32, tag="vals")
            nc.vector.tensor_copy(vals[:], mv3[:, :, A])
            nc.sync.dma_start(val_d[:], vals[:])

    nc.compile()
    return nc


_NC_CACHE = {}


def kernel(obs_seq, actions_seq, dones_seq, h0, c0,
           enc_w1, enc_b1, enc_w2, enc_b2,
           w_ih, w_hh, b_ih, b_hh,
           am_w, am_b, actor_logstd, cr_w, cr_b):
    bf16 = ml_dtypes.bfloat16
    for z_ in (enc_b1, enc_b2, b_ih, b_hh, am_b, cr_b, actor_logstd):
        assert np.allclose(np.asarray(z_), 0.0), "nonzero biases unsupported"

    wih_r = np.asarray(w_ih)   # [4H, H], reference gate order i,f,g,o
    whh_r = np.asarray(w_hh)
    shared = {
        "w1T": np.ascontiguousarray(np.asarray(enc_w1).T).astype(bf16),
        "w2T": np.ascontiguousarray(np.asarray(enc_w2).T).astype(bf16),
        "wihT": np.ascontiguousarray(wih_r.T).astype(bf16),
        "whhT": np.ascontiguousarray(whh_r.T).astype(bf16),
        "amcrT": np.ascontiguousarray(
            np.concatenate([np.asarray(am_w), np.asarray(cr_w)], axis=0).T
        ).astype(bf16),
    }

    obs = np.asarray(obs_seq)
    acts = np.asarray(actions_seq)
    dones = np.asarray(dones_seq)
    h0 = np.asarray(h0)
    c0 = np.asarray(c0)

    in_maps = []
    for s in range(NCORES):
        sl = slice(s * BS, (s + 1) * BS)
        obsT = np.ascontiguousarray(obs[:, sl, :].transpose(2, 0, 1)
                                    ).reshape(128, T * BS)  # [O, T*BS]
        msk = (1.0 - dones[:, sl]).astype(np.float32)  # [T, BS]
        maskb = np.ascontiguousarray(
            np.broadcast_to(msk[:, None, :], (T, 128, BS))).astype(bf16)
        act_nat = np.ascontiguousarray(acts[:, sl, :].transpose(1, 0, 2)
                                       ).reshape(128, T * A).astype(np.float32)
        in_maps.append({
            "obsT": obsT.astype(bf16),
            "maskb": maskb,
            "act_nat": act_nat,
            "h0T": np.ascontiguousarray(h0[sl].T).astype(bf16),
            "c0T": np.ascontiguousarray(c0[sl].T).astype(np.float32),
            **shared,
        })

    if "nc" not in _NC_CACHE:
        _NC_CACHE["nc"] = build_program()
    nc = _NC_CACHE["nc"]

    trace = bool(int(os.environ.get("LSTM_KERNEL_TRACE", "0")))
    res = run_bass_kernel_spmd(nc, in_maps, core_ids=list(range(NCORES)),
                               trace=trace)
    if trace:
        print("HW exec time:", res.exec_time_ns, "ns")
        print("mean exec time:", res.mean_exec_time_ns, "ns")
        _NC_CACHE["last_results"] = res
        try:
            import json
            insts, tpath = res.instructions_and_trace
            rows = [(i.name, i.op_name, i.engine, i.timestamp, i.duration,
                     i.source_line, i.evt_wait_time) for i in insts]
            with open("/root/problem/work/last_insts.json", "w") as f:
                json.dump({"trace_path": tpath, "rows": rows}, f)
            print("trace dumped:", tpath, len(rows), "insts")
        except Exception as e:
            print("trace dump failed:", e)

    logprobs = np.empty((T, B), np.float32)
    entropies = np.empty((T, B), np.float32)
    values = np.empty((T, B), np.float32)
    hT = np.empty((B, H), np.float32)
    cT = np.empty((B, H), np.float32)
    for s in range(NCORES):
        sl = slice(s * BS, (s + 1) * BS)
        r = res.results[s]
        logprobs[:, sl] = r["lp_nat"].T
        entropies[:, sl] = r["ent_nat"].T
        values[:, sl] = r["val_nat"].T
        hT[sl] = r["hT_out"].T
        cT[sl] = r["cT_out"].T
    return logprobs, entropies, values, hT, cT


# revision 41
# speedup vs baseline: 1.0033x; 1.0033x over previous
"""Trainium2 Bass kernel for ActorCriticLSTM (T=256, B=1024, O=H=128, A=8).

Strategy: data-parallel over batch (8 cores x 128 batch). Per core:
  - Encoder (2-layer tanh MLP) computed in "transposed activation" layout
    xT [feature, (t, b)] so chained matmuls need no transposes; obs is
    pre-transposed on host and uploaded as one [128, T*B] bf16 matrix.
  - LSTM recurrence fully unrolled over T=256 steps, state kept transposed
    (hT [H,B] bf16 slice of a persistent ys buffer, cT [H,B] f32).
    Gates for step t are 4 PSUM column blocks [H,B] each computed as
    W_ih_g @ x2_t (start) + W_hh_g @ (mask*h) (accumulate).
    Done-masking uses a host-precomputed broadcast mask [T,128,B] bf16.
  - Actor/critic heads + tanh-normal logprob math in natural layout
    [batch, (t,a)] after the scan; Ln-based atanh; outputs left in
    device-natural layouts and fixed up (transposed) on host.
"""

import os
import numpy as np
import ml_dtypes

import concourse.bass as bass
import concourse.bacc as bacc
import concourse.tile as tile
from concourse import mybir
from concourse.bass_utils import run_bass_kernel_spmd

BF16 = mybir.dt.bfloat16
F32 = mybir.dt.float32
AF = mybir.ActivationFunctionType
ALU = mybir.AluOpType

LOG2PI = float(np.log(2.0 * np.pi))
TANH_EPS = 1e-06

T, B, O, H, A = 256, 1024, 128, 128, 8
NCORES = 8
BS = B // NCORES  # 128 batch per core


def build_program(T_=T, CT=4):
    """Build the single-core SPMD Bass program."""
    nc = bacc.Bacc("TRN2", target_bir_lowering=False, debug=False)

    # ---- DRAM I/O ----
    obsT_d = nc.dram_tensor("obsT", [128, T_ * BS], BF16, kind="ExternalInput")
    mask_d = nc.dram_tensor("maskb", [T_, 128, BS], BF16, kind="ExternalInput")
    act_d = nc.dram_tensor("act_nat", [128, T_ * A], F32, kind="ExternalInput")
    h0T_d = nc.dram_tensor("h0T", [H, BS], BF16, kind="ExternalInput")
    c0T_d = nc.dram_tensor("c0T", [H, BS], F32, kind="ExternalInput")
    w1T_d = nc.dram_tensor("w1T", [O, H], BF16, kind="ExternalInput")
    w2T_d = nc.dram_tensor("w2T", [H, H], BF16, kind="ExternalInput")
    wihT_d = nc.dram_tensor("wihT", [H, 4 * H], BF16, kind="ExternalInput")
    whhT_d = nc.dram_tensor("whhT", [H, 4 * H], BF16, kind="ExternalInput")
    amcr_d = nc.dram_tensor("amcrT", [H, A + 1], BF16, kind="ExternalInput")

    lp_d = nc.dram_tensor("lp_nat", [128, T_], F32, kind="ExternalOutput")
    ent_d = nc.dram_tensor("ent_nat", [128, T_], F32, kind="ExternalOutput")
    val_d = nc.dram_tensor("val_nat", [128, T_], F32, kind="ExternalOutput")
    hT_out_d = nc.dram_tensor("hT_out", [H, BS], F32, kind="ExternalOutput")
    cT_out_d = nc.dram_tensor("cT_out", [H, BS], F32, kind="ExternalOutput")

    NCH = T_ // CT
    with tile.TileContext(nc) as tc:
        with (
            tc.tile_pool(name="const", bufs=1) as cpool,
            tc.tile_pool(name="x2all", bufs=1) as x2pool,
            tc.tile_pool(name="ysall", bufs=1) as yspool,
            tc.tile_pool(name="obs", bufs=3) as obspool,
            tc.tile_pool(name="x1", bufs=2) as x1pool,
            tc.tile_pool(name="epsum", bufs=1, space="PSUM") as epsum,
            tc.tile_pool(name="gpsum", bufs=2, space="PSUM") as gpsum,
            tc.tile_pool(name="hpsum", bufs=2, space="PSUM") as hpsum,
            tc.tile_pool(name="mask", bufs=12) as mpool,
            tc.tile_pool(name="work", bufs=4) as wpool,
            tc.tile_pool(name="cstate", bufs=3) as capool,
            tc.tile_pool(name="post", bufs=1) as ppool,
        ):
            # actions DMA first: it gates L1/L2 which head the ACT queue.
            # Chunked halves so the first Ln ops overlap the second DMA half.
            acts = ppool.tile([128, T_ * A], F32, tag="acts")
            L1 = ppool.tile([128, T_ * A], F32, tag="L1")
            L2 = ppool.tile([128, T_ * A], F32, tag="L2")
            half = T_ * A // 2
            for q in range(2):
                qs = slice(q * half, (q + 1) * half)
                nc.sync.dma_start(acts[:, qs], act_d[:, qs])
            for q in range(2):
                qs = slice(q * half, (q + 1) * half)
                nc.scalar.activation(L1[:, qs], acts[:, qs], AF.Ln,
                                     bias=1.0, scale=1.0)
                nc.scalar.activation(L2[:, qs], acts[:, qs], AF.Ln,
                                     bias=1.0, scale=-1.0)

            # ---- constants ----
            w1T = cpool.tile([O, H], BF16, tag="w1T")
            nc.sync.dma_start(w1T[:], w1T_d[:])
            w2T = cpool.tile([H, H], BF16, tag="w2T")
            nc.sync.dma_start(w2T[:], w2T_d[:])
            wihT = cpool.tile([H, 4 * H], BF16, tag="wihT")
            nc.sync.dma_start(wihT[:], wihT_d[:])
            whhT = cpool.tile([H, 4 * H], BF16, tag="whhT")
            nc.sync.dma_start(whhT[:], whhT_d[:])
            amcr = cpool.tile([H, A + 1], BF16, tag="amcr")
            nc.sync.dma_start(amcr[:], amcr_d[:])
            h0T = cpool.tile([H, BS], BF16, tag="h0T")
            nc.sync.dma_start(h0T[:], h0T_d[:])
            c0T = cpool.tile([H, BS], F32, tag="c0T")
            nc.sync.dma_start(c0T[:], c0T_d[:])

            x2all = x2pool.tile([128, T_ * BS], BF16, tag="x2all")
            ysall = yspool.tile([128, T_ * BS], BF16, tag="ysall")
            mv = ppool.tile([128, T_ * (A + 1)], F32, tag="mv")

            # logprob math is cut into [128,256] pieces and spread through
            # the recurrence (DVE is ~50% idle there); tile reuse:
            # d -> ljbuf (piecewise, after its ljs reduce), z -> L1, zsq -> L2
            ljbuf = ppool.tile([128, T_ * A], F32, tag="ljbuf")
            ljs = ppool.tile([128, T_], F32, tag="ljs")
            zsum = ppool.tile([128, T_], F32, tag="zsum")
            mv3 = mv[:].rearrange("p (t n) -> p t n", n=A + 1)
            NP = max(1, T_ * A // 256)  # pieces of 256 cols = 32 steps
            SP = T_ // NP

            PW = T_ * A // NP

            def pc(x, k):
                return x[:, k * PW:(k + 1) * PW]

            def lj_piece(k):
                nc.vector.tensor_add(pc(ljbuf, k), pc(L1, k), pc(L2, k))

            def ljs_piece(k):
                nc.vector.tensor_reduce(
                    ljs[:, k * SP:(k + 1) * SP],
                    pc(ljbuf, k).rearrange("p (t a) -> p t a", a=A),
                    axis=mybir.AxisListType.X, op=ALU.add)

            def d_piece(k):
                nc.vector.tensor_sub(pc(ljbuf, k), pc(L1, k), pc(L2, k))

            def z_piece(k):
                nc.vector.scalar_tensor_tensor(
                    pc(L1, k).rearrange("p (t a) -> p t a", a=A),
                    pc(ljbuf, k).rearrange("p (t a) -> p t a", a=A), 0.5,
                    mv3[:, k * SP:(k + 1) * SP, 0:A],
                    op0=ALU.mult, op1=ALU.subtract)

            def zsq_piece(k):
                nc.vector.tensor_mul(pc(L2, k), pc(L1, k), pc(L1, k))

            def zsum_piece(k):
                nc.vector.tensor_reduce(
                    zsum[:, k * SP:(k + 1) * SP],
                    pc(L2, k).rearrange("p (t a) -> p t a", a=A),
                    axis=mybir.AxisListType.X, op=ALU.add)

            spread = {}
            for k in range(NP):
                spread.setdefault(8 + 3 * k, []).extend(
                    [lambda k=k: lj_piece(k), lambda k=k: ljs_piece(k),
                     lambda k=k: d_piece(k)])
            for k in range(NP - 1):
                s = SP * (k + 1) + 2
                spread.setdefault(s, []).append(lambda k=k: z_piece(k))
                spread.setdefault(s + 1, []).append(lambda k=k: zsq_piece(k))
                spread.setdefault(s + 2, []).append(lambda k=k: zsum_piece(k))

            # Per-step recurrence state (python-side references):
            #   cm[t]    = c(t-1) * m(t)          (GpSimd, off critical path)
            #   som[t-1] = sigma_o(t-1) * m(t)    (GpSimd, off critical path)
            #   hm[t]    = som[t-1] * th(t-1)     (DVE, critical)
            # Critical cycle: hm -> hh-mms -> sigma_if -> t1 -> cnew -> tanh -> hm
            mtiles = {}

            def mask_tile(t):
                if t not in mtiles and t < T_:
                    m = mpool.tile([128, BS], BF16, tag="mask")
                    nc.sync.dma_start(m[:], mask_d[t])
                    mtiles[t] = m
                return mtiles.get(t)

            cm_t = None      # c(t-1)*m(t) for current step
            th_prev = None   # tanh(c(t-1))
            sig_prev = None  # sigma tile of step t-1 (for sigma_o)
            som_prev = None  # sigma_o(t-1)*m(t)
            gtiles = {}      # step -> gate psum tile (ih-mms pre-issued)

            def issue_ih(t):
                if t >= T_ or t in gtiles:
                    return
                # two banks: (i,f) and (g,o); sigma_if can then start as
                # soon as the i,f hh-mms land, without waiting for g,o
                gif = gpsum.tile([128, 256], F32, tag="gif")
                ggo = gpsum.tile([128, 256], F32, tag="ggo")
                x2t = x2all[:, t * BS:(t + 1) * BS]
                for gi in range(4):
                    # start=True on the first mm zero-marks the whole bank;
                    # the second ih-mm overwrites its (pending-zero) slice,
                    # hh-mms then accumulate; last hh-mm closes the group.
                    gt = gif if gi < 2 else ggo
                    nc.tensor.matmul(gt[:, (gi % 2) * 128:(gi % 2 + 1) * 128],
                                     wihT[:, gi * 128:(gi + 1) * 128], x2t,
                                     start=(gi % 2 == 0), stop=False)
                gtiles[t] = (gif, ggo)

            enc_state = {}

            def encoder_piece(c, phase, after=None):
                # one piece per recurrence step: phase 0 = obs DMA,
                # 1 = layer-1 mm+tanh, 2 = layer-2 mm+tanh (CT*BS == 512)
                if c >= NCH or phase > 2:
                    return
                if phase == 0:
                    obst = obspool.tile([128, CT * BS], BF16, tag="obs")
                    nc.gpsimd.dma_start(
                        obst[:], obsT_d[:, c * CT * BS:(c + 1) * CT * BS])
                    enc_state[c] = obst
                    return
                # negative priority offset: encoder work appears ~2 steps
                # later to the scheduler, so its tanh lands in ACT idle gaps
                # instead of between chain ACT ops (hard dep-pinning stalls
                # the in-order ACT queue; this is only a placement hint)
                if phase == 1:
                    obst = enc_state[c]
                    x1 = x1pool.tile([128, CT * BS], BF16, tag="x1")
                    with tc.high_priority(offset=-60):
                        ep = epsum.tile([128, 512], F32, tag="ep")
                        nc.tensor.matmul(ep[:], w1T[:], obst[:],
                                         start=True, stop=True)
                        nc.scalar.activation(x1[:], ep[:], AF.Tanh)
                    enc_state[c] = x1
                else:
                    x1 = enc_state.pop(c)
                    with tc.high_priority(offset=-60):
                        ep2 = epsum.tile([128, 512], F32, tag="ep")
                        nc.tensor.matmul(ep2[:], w2T[:], x1[:],
                                         start=True, stop=True)
                        nc.scalar.activation(
                            x2all[:, c * CT * BS:(c + 1) * CT * BS],
                            ep2[:], AF.Tanh)
                del after

            def encoder_chunk(c):
                for ph in range(3):
                    encoder_piece(c, ph)

            cprev = c0T
            # encoder runs one chunk ahead of the recurrence so the ih-mm
            # lookahead never stalls at chunk boundaries
            encoder_chunk(0)
            encoder_chunk(1)
            issue_ih(0)
            for c in range(NCH):
                # ---------- recurrence steps ----------
                for t in range(c * CT, (c + 1) * CT):
                    m_cur = mask_tile(t)
                    m_next = mask_tile(t + 1)
                    mask_tile(t + 6)  # deepen DMA prefetch
                    issue_ih(t + 1)
                    if t == 0:
                        hm = wpool.tile([H, BS], BF16, tag="hm")
                        nc.vector.tensor_mul(hm[:], h0T[:], m_cur[:])
                        cm_t = wpool.tile([H, BS], F32, tag="cm0")
                        nc.gpsimd.tensor_mul(cm_t[:], c0T[:], m_cur[:])
                    else:
                        # hm = (sigma_o(t-1)*m(t)) * tanh(c(t-1))  [critical]
                        hm = wpool.tile([H, BS], BF16, tag="hm")
                        nc.vector.tensor_mul(hm[:], som_prev[:], th_prev[:])
                        # ys[t-1] = sigma_o(t-1) * tanh(c(t-1))    [off-path]
                        ysl_prev = ysall[:, (t - 1) * BS:t * BS]
                        nc.vector.tensor_mul(ysl_prev, sig_prev[:, 384:512],
                                             th_prev[:])
                    gif, ggo = gtiles.pop(t)
                    for gi in range(4):
                        gt = gif if gi < 2 else ggo
                        nc.tensor.matmul(gt[:, (gi % 2) * 128:(gi % 2 + 1) * 128],
                                         whhT[:, gi * 128:(gi + 1) * 128], hm[:],
                                         start=False, stop=(gi % 2 == 1))
                    if t > 0:
                        # heads for t-1, after the critical hh-mms
                        hp = hpsum.tile([128, A + 1], F32, tag="hp")
                        nc.tensor.matmul(hp[:], ysall[:, (t - 1) * BS:t * BS],
                                         amcr[:], start=True, stop=True)
                        nc.vector.tensor_copy(
                            mv[:, (t - 1) * (A + 1):t * (A + 1)], hp[:])
                    # gate order i,f,g,o (reference order)
                    sig = wpool.tile([128, 4 * H], BF16, tag="sig")
                    nc.scalar.activation(sig[:, 0:256], gif[:], AF.Sigmoid)
                    tg = wpool.tile([128, H], BF16, tag="tg")
                    nc.scalar.activation(tg[:], ggo[:, 0:128], AF.Tanh)
                    nc.scalar.activation(sig[:, 384:512], ggo[:, 128:256], AF.Sigmoid)
                    # c-path on DVE: t2 first (frees the cnew issue as soon
                    # as t1 lands), t1 after tanh_g
                    t2 = wpool.tile([H, BS], F32, tag="t2")
                    nc.vector.tensor_mul(t2[:], sig[:, 128:256], cm_t[:])
                    t1 = wpool.tile([H, BS], BF16, tag="t1")
                    nc.vector.tensor_mul(t1[:], sig[:, 0:128], tg[:])
                    cnew = capool.tile([H, BS], F32, tag="c")
                    nc.vector.tensor_add(cnew[:], t1[:], t2[:])
                    th = wpool.tile([H, BS], BF16, tag="th")
                    th_i = nc.scalar.activation(th[:], cnew[:], AF.Tanh)
                    if t + 1 < T_:
                        cm_t = wpool.tile([H, BS], F32, tag="cmn")
                        nc.gpsimd.tensor_mul(cm_t[:], cnew[:], m_next[:])
                        som_prev = wpool.tile([H, BS], BF16, tag="som")
                        nc.gpsimd.tensor_mul(som_prev[:], sig[:, 384:512],
                                             m_next[:])
                    th_prev = th
                    sig_prev = sig
                    cprev = cnew
                    encoder_piece(c + 2, t % CT, after=th_i)
                    for fn in spread.pop(t, []):
                        fn()

            # tail: ys / heads for the last step
            ysl_last = ysall[:, (T_ - 1) * BS:T_ * BS]
            nc.vector.tensor_mul(ysl_last, sig_prev[:, 384:512], th_prev[:])
            hp = hpsum.tile([128, A + 1], F32, tag="hp")
            nc.tensor.matmul(hp[:], ysl_last, amcr[:], start=True, stop=True)
            nc.vector.tensor_copy(mv[:, (T_ - 1) * (A + 1):T_ * (A + 1)], hp[:])

            # ---------- final state outputs ----------
            nc.sync.dma_start(cT_out_d[:], cprev[:])
            hT_f32 = ppool.tile([H, BS], F32, tag="hTf")
            nc.vector.tensor_copy(hT_f32[:], ysall[:, (T_ - 1) * BS:T_ * BS])
            nc.sync.dma_start(hT_out_d[:], hT_f32[:])

            # ---------- logprob finals (last piece + reductions) ----------
            z_piece(NP - 1)
            zsq_piece(NP - 1)
            zsum_piece(NP - 1)
            lp = ppool.tile([128, T_], F32, tag="lp")
            nc.vector.scalar_tensor_tensor(lp[:], zsum[:], -0.5, ljs[:],
                                           op0=ALU.mult, op1=ALU.subtract)
            lpf = ppool.tile([128, T_], F32, tag="lpf")
            nc.vector.tensor_scalar_add(lpf[:], lp[:], -0.5 * A * LOG2PI)
            nc.sync.dma_start(lp_d[:], lpf[:])
            ent = ppool.tile([128, T_], F32, tag="ent")
            nc.vector.tensor_scalar_add(ent[:], ljs[:], A * (0.5 + 0.5 * LOG2PI))
            nc.sync.dma_start(ent_d[:], ent[:])
            vals = ppool.tile([128, T_], F32, tag="vals")
            nc.vector.tensor_copy(vals[:], mv3[:, :, A])
            nc.sync.dma_start(val_d[:], vals[:])

    nc.compile()
    return nc


_NC_CACHE = {}


def kernel(obs_seq, actions_seq, dones_seq, h0, c0,
           enc_w1, enc_b1, enc_w2, enc_b2,
           w_ih, w_hh, b_ih, b_hh,
           am_w, am_b, actor_logstd, cr_w, cr_b):
    bf16 = ml_dtypes.bfloat16
    for z_ in (enc_b1, enc_b2, b_ih, b_hh, am_b, cr_b, actor_logstd):
        assert np.allclose(np.asarray(z_), 0.0), "nonzero biases unsupported"

    wih_r = np.asarray(w_ih)   # [4H, H], reference gate order i,f,g,o
    whh_r = np.asarray(w_hh)
    shared = {
        "w1T": np.ascontiguousarray(np.asarray(enc_w1).T).astype(bf16),
        "w2T": np.ascontiguousarray(np.asarray(enc_w2).T).astype(bf16),
        "wihT": np.ascontiguousarray(wih_r.T).astype(bf16),
        "whhT": np.ascontiguousarray(whh_r.T).astype(bf16),
        "amcrT": np.ascontiguousarray(
            np.concatenate([np.asarray(am_w), np.asarray(cr_w)], axis=0).T
        ).astype(bf16),
    }

    obs = np.asarray(obs_seq)
    acts = np.asarray(actions_seq)
    dones = np.asarray(dones_seq)
    h0 = np.asarray(h0)
    c0 = np.asarray(c0)

    in_maps = []
    for s in range(NCORES):
        sl = slice(s * BS, (s + 1) * BS)
        obsT = np.ascontiguousarray(obs[:, sl, :].transpose(2, 0, 1)
                                    ).reshape(128, T * BS)  # [O, T*BS]
        msk = (1.0 - dones[:, sl]).astype(np.float32)  # [T, BS]
        maskb = np.ascontiguousarray(
            np.broadcast_to(msk[:, None, :], (T, 128, BS))).astype(bf16)
        act_nat = np.ascontiguousarray(acts[:, sl, :].transpose(1, 0, 2)
                                       ).reshape(128, T * A).astype(np.float32)
        in_maps.append({
            "obsT": obsT.astype(bf16),
            "maskb": maskb,
            "act_nat": act_nat,
            "h0T": np.ascontiguousarray(h0[sl].T).astype(bf16),
            "c0T": np.ascontiguousarray(c0[sl].T).astype(np.float32),
            **shared,
        })

    if "nc" not in _NC_CACHE:
        _NC_CACHE["nc"] = build_program()
    nc = _NC_CACHE["nc"]

    trace = bool(int(os.environ.get("LSTM_KERNEL_TRACE", "0")))
    res = run_bass_kernel_spmd(nc, in_maps, core_ids=list(range(NCORES)),
                               trace=trace)
    if trace:
        print("HW exec time:", res.exec_time_ns, "ns")
        print("mean exec time:", res.mean_exec_time_ns, "ns")
        _NC_CACHE["last_results"] = res
        try:
            import json
            insts, tpath = res.instructions_and_trace
            rows = [(i.name, i.op_name, i.engine, i.timestamp, i.duration,
                     i.source_line, i.evt_wait_time) for i in insts]
            with open("/root/problem/work/last_insts.json", "w") as f:
                json.dump({"trace_path": tpath, "rows": rows}, f)
            print("trace dumped:", tpath, len(rows), "insts")
        except Exception as e:
            print("trace dump failed:", e)

    logprobs = np.empty((T, B), np.float32)
    entropies = np.empty((T, B), np.float32)
    values = np.empty((T, B), np.float32)
    hT = np.empty((B, H), np.float32)
    cT = np.empty((B, H), np.float32)
    for s in range(NCORES):
        sl = slice(s * BS, (s + 1) * BS)
        r = res.results[s]
        logprobs[:, sl] = r["lp_nat"].T
        entropies[:, sl] = r["ent_nat"].T
        values[:, sl] = r["val_nat"].T
        hT[sl] = r["hT_out"].T
        cT[sl] = r["cT_out"].T
    return logprobs, entropies, values, hT, cT
